# revision 22
# baseline (speedup 1.0000x reference)
# Emu3 VQVAE vector-quantizer kernel for 8x TRN2 NeuronCores (Bass/Tile).
#
# distances = |x|^2 + |e|^2 - 2 x.e ; argmin over K=16384 with first-index
# ties; d = fl(xsq - 2mm) must reproduce the reference's fp32 quantization
# (~4% of rows have exact ties at the min).
#
# Sharding: data-parallel over the 8 batch entries (1024 tokens each);
# codebook replicated.
#
# PE: float32r single pass (1 cyc/row at moving dim >= 256; ~5/8192 flipped
# indices vs exact fp32, rel-err 6.7e-4, validated on HW). bf16x3 fallback.
#
# The kernel is DVE-bound, so per-slab work is spread across three engines.
# Per (tile, section) slab [128 tok x 2048 codes]:
#   ACT: d = fl(xsq - 2mm), one 512-wide op per PSUM bank.
#   keys, per section code in PACK:
#     "s": DVE scalar_tensor_tensor  keys = (d - base) + k*ulp/2048
#          (one fused op; X2_t = iota*ulp_t/2048 materialized once per tile;
#          all terms exact powers of two)
#     "A": ACT keys0 = S*d + (-S*base) (exact), then DVE tt + iota
#     "P": ACT keys0, then GpSimd tt + iota
#   DVE tensor_reduce min -> minik[tile*8+sec]
#   ("s" keys are in ulp/2048 units; rescaled exactly during decode)
# Batched decode over all 64 minima at the end (bitwise dq/k split,
# cross-section min via keys2 = dq*8 + s, masked select).

import numpy as np

B, T, C, H, W = 8, 1, 256, 32, 32
K = 16384
NCORES = 8
NTOK = H * W
NTILES = NTOK // 128
CHUNK = 512
SECW = 2048
NSECT = K // SECW
TCOLS = NTILES * NSECT

_CACHE = {}


def _build_bass(matmul_dtype_name="float32r", repeats=1, pack=("s",) * 8):
    from contextlib import ExitStack

    import concourse.bass as bass  # noqa: F401
    import concourse.mybir as mybir
    import concourse.tile as tile
    from concourse import bacc

    f32 = mybir.dt.float32
    bf16 = mybir.dt.bfloat16
    is_bf16x3 = matmul_dtype_name == "bf16x3"
    mm_dt = bf16 if is_bf16x3 else getattr(mybir.dt, matmul_dtype_name)
    i32 = mybir.dt.int32
    AF = mybir.ActivationFunctionType
    ALU = mybir.AluOpType

    any_stt = any(p == "s" for p in pack)

    nc = bacc.Bacc(
        "TRN2",
        target_bir_lowering=False,
        debug=False,
        enable_asserts=False,
        num_devices=NCORES,
    )

    NS = 2 if is_bf16x3 else 1
    xT_d = nc.dram_tensor("xT", (NS, 2, 128, NTOK), mm_dt, kind="ExternalInput").ap()
    cb_d = nc.dram_tensor("cbT2", (NS, 2, 128, K), mm_dt, kind="ExternalInput").ap()
    xsq_d = nc.dram_tensor("xsqp", (128, NTILES), f32, kind="ExternalInput").ap()
    base_d = nc.dram_tensor("base", (128, NTILES), f32, kind="ExternalInput").ap()
    scal_d = nc.dram_tensor("scal", (128, NTILES), f32, kind="ExternalInput").ap()
    nbs_d = nc.dram_tensor("nbaseS", (128, NTILES), f32, kind="ExternalInput").ap()
    iot_d = nc.dram_tensor(
        "iotas", (128, SECW + 2 * TCOLS), f32, kind="ExternalInput"
    ).ap()
    pmul_d = nc.dram_tensor("pmul", (128, NTILES), f32, kind="ExternalInput").ap()
    scalb_d = nc.dram_tensor("scalb", (128, TCOLS), f32, kind="ExternalInput").ap()
    out_d = nc.dram_tensor("idx", (NTILES, 128, 1), i32, kind="ExternalOutput").ap()

    with tile.TileContext(nc) as tc:
        with ExitStack() as ctx:
            cbp = ctx.enter_context(tc.tile_pool(name="cb", bufs=1))
            xp = ctx.enter_context(tc.tile_pool(name="x", bufs=1))
            sp = ctx.enter_context(tc.tile_pool(name="slab", bufs=4))
            x2p = ctx.enter_context(tc.tile_pool(name="x2", bufs=2))
            pp = ctx.enter_context(tc.tile_pool(name="psum", bufs=8, space="PSUM"))
            smp = ctx.enter_context(tc.tile_pool(name="small", bufs=2))
            outp = ctx.enter_context(tc.tile_pool(name="outs", bufs=2))

            xts = {}
            for hl in range(NS):
                for cs in range(2):
                    xt = xp.tile([128, NTOK], mm_dt, tag=f"x{hl}_{cs}")
                    nc.sync.dma_start(xt[:], xT_d[hl][cs])
                    xts[hl, cs] = xt
            xsq = xp.tile([128, NTILES], f32, tag="xsq")
            nc.sync.dma_start(xsq[:], xsq_d[:])
            base = xp.tile([128, NTILES], f32, tag="base")
            nc.sync.dma_start(base[:], base_d[:])
            scal = xp.tile([128, NTILES], f32, tag="scal")
            nc.sync.dma_start(scal[:], scal_d[:])
            nbs = xp.tile([128, NTILES], f32, tag="nbs")
            nc.sync.dma_start(nbs[:], nbs_d[:])
            pmul = xp.tile([128, NTILES], f32, tag="pmul")
            nc.sync.dma_start(pmul[:], pmul_d[:])
            scalb = xp.tile([128, TCOLS], f32, tag="scalb")
            nc.sync.dma_start(scalb[:], scalb_d[:])

            cbs = {}
            for hl in range(NS):
                for cs in range(2):
                    cbt = cbp.tile([128, K], mm_dt, tag=f"cb{hl}_{cs}")
                    nc.sync.dma_start(cbt[:], cb_d[hl][cs])
                    cbs[hl, cs] = cbt

            iotas = xp.tile([128, SECW + 2 * TCOLS], f32, tag="iotas")
            nc.sync.dma_start(iotas[:], iot_d[:])
            iota_sec = iotas[:, 0:SECW]
            iota_s64 = iotas[:, SECW : SECW + TCOLS]
            iota_w64 = iotas[:, SECW + TCOLS : SECW + 2 * TCOLS]
            c_klo = xp.tile([128, TCOLS], i32, tag="c_klo")
            nc.vector.memset(c_klo[:], SECW - 1)
            c_khi = xp.tile([128, TCOLS], i32, tag="c_khi")
            nc.vector.memset(c_khi[:], -SECW)
            c_s = xp.tile([128, NTILES], i32, tag="c_s")
            nc.vector.memset(c_s[:], NSECT - 1)

            if is_bf16x3:
                TERMS = [(0, 0, 0), (0, 0, 1), (0, 1, 0), (0, 1, 1), (1, 0, 0), (1, 0, 1)]
            else:
                TERMS = [(0, 0, 0), (0, 0, 1)]

            for rep in range(repeats):
                minik = smp.tile([128, TCOLS], f32, tag="minik")
                for t in range(NTILES):
                    if any_stt:
                        # X2_t[p,k] = k * ulp_t[p]/2048, exact pow2*int
                        x2t = x2p.tile([128, SECW], f32, tag="x2t")
                        nc.vector.tensor_scalar(
                            x2t[:],
                            iota_sec,
                            pmul[:, t : t + 1],
                            None,
                            op0=ALU.mult,
                        )
                    for sec in range(NSECT):
                        col = t * NSECT + sec
                        pss = [
                            pp.tile(
                                [128, CHUNK], f32, tag="ps",
                                name=f"ps_{rep}_{t}_{sec}_{ci}",
                            )
                            for ci in range(SECW // CHUNK)
                        ]
                        for ti, (xhl, ehl, cs) in enumerate(TERMS):
                            for ci in range(SECW // CHUNK):
                                k0 = sec * SECW + ci * CHUNK
                                nc.tensor.matmul(
                                    pss[ci][:],
                                    xts[xhl, cs][:, t * 128 : (t + 1) * 128],
                                    cbs[ehl, cs][:, k0 : k0 + CHUNK],
                                    start=(ti == 0),
                                    stop=(ti == len(TERMS) - 1),
                                )
                        slab = sp.tile([128, SECW], f32, tag="slab")
                        # d = fl(xsq - 2mm); per-bank ACT so each chunk's d
                        # starts as soon as its matmul pair retires
                        for ci in range(SECW // CHUNK):
                            nc.scalar.activation(
                                slab[:, ci * CHUNK : (ci + 1) * CHUNK],
                                pss[ci][:],
                                AF.Identity,
                                bias=xsq[:, t : t + 1],
                                scale=-1.0,
                            )
                        if pack[sec] == "s":
                            # fused keys = (d - base) + k*ulp/2048 (exact)
                            nc.vector.scalar_tensor_tensor(
                                slab[:],
                                slab[:],
                                base[:, t : t + 1],
                                x2t[:],
                                op0=ALU.subtract,
                                op1=ALU.add,
                            )
                        else:
                            # ACT: keys0 = S*d + (-S*base), exact; then +iota
                            nc.scalar.activation(
                                slab[:],
                                slab[:],
                                AF.Identity,
                                bias=nbs[:, t : t + 1],
                                scale=scal[:, t : t + 1],
                            )
                            eng = nc.vector if pack[sec] == "A" else nc.gpsimd
                            eng.tensor_tensor(
                                slab[:], slab[:], iota_sec, op=ALU.add
                            )
                        nc.vector.tensor_reduce(
                            minik[:, col : col + 1],
                            slab[:],
                            axis=mybir.AxisListType.X,
                            op=ALU.min,
                        )

                # ---- batched decode over all 64 (tile,sec) minima ----
                if any_stt:
                    # rescale "s" columns to integer keys; exact pow2 for
                    # "s", and for A/P columns scalb holds 1.0
                    nc.vector.tensor_tensor(
                        minik[:], minik[:], scalb[:], op=ALU.mult
                    )
                minik_i = smp.tile([128, TCOLS], i32, tag="minik_i")
                nc.vector.tensor_copy(minik_i[:], minik[:])
                kmod_i = smp.tile([128, TCOLS], i32, tag="kmod_i")
                nc.vector.tensor_tensor(
                    kmod_i[:], minik_i[:], c_klo[:], op=ALU.bitwise_and
                )
                kmod = smp.tile([128, TCOLS], f32, tag="kmod")
                nc.vector.tensor_copy(kmod[:], kmod_i[:])
                dqw_i = smp.tile([128, TCOLS], i32, tag="dqw_i")
                nc.vector.tensor_tensor(
                    dqw_i[:], minik_i[:], c_khi[:], op=ALU.bitwise_and
                )
                dqw = smp.tile([128, TCOLS], f32, tag="dqw")
                nc.vector.tensor_copy(dqw[:], dqw_i[:])
                keys2 = smp.tile([128, TCOLS], f32, tag="keys2")
                nc.vector.tensor_scalar(
                    keys2[:], dqw[:], float(NSECT) / float(SECW), None, op0=ALU.mult
                )
                nc.vector.tensor_tensor(keys2[:], keys2[:], iota_s64, op=ALU.add)
                kfull = smp.tile([128, TCOLS], f32, tag="kfull")
                nc.vector.tensor_tensor(kfull[:], iota_w64, kmod[:], op=ALU.add)
                m2 = smp.tile([128, NTILES], f32, tag="m2")
                nc.vector.tensor_reduce(
                    m2[:],
                    keys2[:].rearrange("p (t s) -> p t s", s=NSECT),
                    axis=mybir.AxisListType.X,
                    op=ALU.min,
                )
                m2i = smp.tile([128, NTILES], i32, tag="m2i")
                nc.vector.tensor_copy(m2i[:], m2[:])
                sstar_i = smp.tile([128, NTILES], i32, tag="sstar_i")
                nc.vector.tensor_tensor(
                    sstar_i[:], m2i[:], c_s[:], op=ALU.bitwise_and
                )
                sstar = smp.tile([128, NTILES], f32, tag="sstar")
                nc.vector.tensor_copy(sstar[:], sstar_i[:])
                mask = smp.tile([128, TCOLS], f32, tag="mask")
                nc.vector.tensor_tensor(
                    mask[:].rearrange("p (t s) -> p t s", s=NSECT),
                    iota_s64.rearrange("p (t s) -> p t s", s=NSECT),
                    sstar[:].unsqueeze(2).broadcast_to([128, NTILES, NSECT]),
                    op=ALU.is_equal,
                )
                nc.vector.tensor_tensor(kfull[:], kfull[:], mask[:], op=ALU.mult)
                kwin = outp.tile([128, NTILES], f32, tag="kwin")
                nc.vector.tensor_reduce(
                    kwin[:],
                    kfull[:].rearrange("p (t s) -> p t s", s=NSECT),
                    axis=mybir.AxisListType.X,
                    op=ALU.add,
                )
                winI = outp.tile([128, NTILES], i32, tag="winI")
                nc.vector.tensor_copy(winI[:], kwin[:])
                for t in range(NTILES):
                    nc.sync.dma_start(out_d[t], winI[:, t : t + 1])

    nc.compile()
    return nc


def get_nc(matmul_dtype_name="float32r", repeats=1, pack=None):
    if pack is None:
        pack = PACK
    key = ("nc", matmul_dtype_name, repeats, tuple(pack))
    if key not in _CACHE:
        _CACHE[key] = _build_bass(matmul_dtype_name, repeats, tuple(pack))
    return _CACHE[key]


def prepare_inputs(hidden_state, codebook, mode="float32r", pack=None):
    """Host-side shard prep: returns in_maps (list of 8 dicts)."""
    import ml_dtypes

    if pack is None:
        pack = PACK
    hs = np.ascontiguousarray(np.asarray(hidden_state, dtype=np.float32))
    cb = np.ascontiguousarray(np.asarray(codebook, dtype=np.float32))
    xT = hs.reshape(B, C, NTOK)
    cb2 = (2.0 * cb.T).astype(np.float32)
    if mode == "bf16x3":
        cb2h = cb2.astype(ml_dtypes.bfloat16)
        cb2l = (cb2 - cb2h.astype(np.float32)).astype(ml_dtypes.bfloat16)
        cb_in = np.ascontiguousarray(np.stack([cb2h, cb2l]).reshape(2, 2, 128, K))
    else:
        cb_in = np.ascontiguousarray(cb2.reshape(1, 2, 128, K))

    sec_ids = np.arange(TCOLS, dtype=np.float32) % NSECT
    iota_row = np.concatenate(
        [np.arange(SECW, dtype=np.float32), sec_ids, sec_ids * SECW]
    )
    iotas = np.ascontiguousarray(np.broadcast_to(iota_row, (128, iota_row.size)))

    emax = float(np.max(np.linalg.norm(2.0 * cb.astype(np.float64), axis=1)))

    # decode rescale: "s" sections produced keys in ulp/2048 units
    s_mask = np.array([1.0 if pack[s] == "s" else 0.0 for s in range(NSECT)],
                      dtype=np.float32)

    in_maps = []
    for b in range(B):
        xb32 = xT[b]
        if mode == "bf16x3":
            xh = xb32.astype(ml_dtypes.bfloat16)
            xl = (xb32 - xh.astype(np.float32)).astype(ml_dtypes.bfloat16)
            xin = np.ascontiguousarray(np.stack([xh, xl]).reshape(2, 2, 128, NTOK))
        else:
            xin = np.ascontiguousarray(xb32.reshape(1, 2, 128, NTOK))
        xsq = np.sum(xb32 * xb32, axis=0, dtype=np.float32)

        xsq64 = xsq.astype(np.float64)
        bound = np.sqrt(xsq64) * emax * 1.2 + 1e-6
        base = (xsq64 - bound).astype(np.float32)
        _, exp = np.frexp(base)
        ulp = np.ldexp(np.float64(1.0), exp - 24)
        dq_max = (xsq64 + bound - base.astype(np.float64)) / ulp
        assert (base > 0).all() and (dq_max < 8100).all(), (
            f"distance-spread exceeds 13-bit key budget; max dq={dq_max.max():.0f}"
        )
        scal = np.ldexp(np.float32(SECW), -(exp - 24)).astype(np.float32)
        nbaseS = (-(base.astype(np.float64) * scal.astype(np.float64))).astype(
            np.float32
        )

        def pt(a):
            return np.ascontiguousarray(a.reshape(NTILES, 128).T)

        scal_t = pt(scal)
        # scalb: per minik column; scal for "s" sections, 1.0 for A/P
        scalb = np.repeat(scal_t, NSECT, axis=1) * s_mask[None, :].repeat(
            NTILES, axis=0
        ).reshape(1, TCOLS)
        scalb = (scalb + (1.0 - np.tile(s_mask, NTILES))[None, :]).astype(np.float32)

        in_maps.append(
            {
                "xT": xin,
                "cbT2": cb_in,
                "xsqp": pt(xsq),
                "base": pt(base),
                "scal": scal_t,
                "nbaseS": pt(nbaseS),
                "iotas": iotas,
                "pmul": np.ascontiguousarray(1.0 / scal_t),
                "scalb": np.ascontiguousarray(scalb),
            }
        )
    return in_maps


MODE = "float32r"
# per-section key engine: "s"=DVE fused stt, "A"=ACT pack + DVE iota-add,
# "P"=ACT pack + GpSimd iota-add
PACK = ("s", "s", "s", "s", "P", "P", "P", "P")


def kernel(hidden_state, codebook):
    from concourse.bass_utils import run_bass_kernel_spmd

    nc = get_nc(MODE, 1, PACK)
    in_maps = prepare_inputs(hidden_state, codebook, MODE, PACK)
    res = run_bass_kernel_spmd(nc, in_maps, core_ids=list(range(NCORES)))
    out = np.stack(
        [res.results[b]["idx"].reshape(NTOK) for b in range(B)], axis=0
    ).astype(np.int32)
    return out.reshape(B, T, H, W)


# revision 23
# speedup vs baseline: 1.0731x; 1.0731x over previous
# Emu3 VQVAE vector-quantizer kernel for 8x TRN2 NeuronCores (Bass/Tile).
#
# distances = |x|^2 + |e|^2 - 2 x.e ; argmin over K=16384 with first-index
# ties; d = fl(xsq - 2mm) must reproduce the reference's fp32 quantization
# (~4% of rows have exact ties at the min).
#
# Sharding: data-parallel over the 8 batch entries (1024 tokens each);
# codebook replicated.
#
# PE: float32r single pass (1 cyc/row at moving dim >= 256; ~5/8192 flipped
# indices vs exact fp32, rel-err 6.7e-4, validated on HW). bf16x3 fallback.
#
# The kernel is DVE-bound, so per-slab work is spread across three engines.
# Per (tile, section) slab [128 tok x 2048 codes]:
#   ACT: d = fl(xsq - 2mm), one 512-wide op per PSUM bank.
#   keys, per section code in PACK:
#     "s": DVE scalar_tensor_tensor  keys = (d - base) + k*ulp/2048
#          (one fused op; X2_t = iota*ulp_t/2048 materialized once per tile;
#          all terms exact powers of two)
#     "A": ACT keys0 = S*d + (-S*base) (exact), then DVE tt + iota
#     "P": ACT keys0, then GpSimd tt + iota
#   DVE tensor_reduce min -> minik[tile*8+sec]
#   ("s" keys are in ulp/2048 units; rescaled exactly during decode)
# Batched decode over all 64 minima at the end (bitwise dq/k split,
# cross-section min via keys2 = dq*8 + s, masked select).

import numpy as np

B, T, C, H, W = 8, 1, 256, 32, 32
K = 16384
NCORES = 8
NTOK = H * W
NTILES = NTOK // 128
CHUNK = 512
SECW = 2048
NSECT = K // SECW
TCOLS = NTILES * NSECT

_CACHE = {}


def _build_bass(matmul_dtype_name="float32r", repeats=1, pack=("s",) * 8):
    from contextlib import ExitStack

    import concourse.bass as bass  # noqa: F401
    import concourse.mybir as mybir
    import concourse.tile as tile
    from concourse import bacc

    f32 = mybir.dt.float32
    bf16 = mybir.dt.bfloat16
    is_bf16x3 = matmul_dtype_name == "bf16x3"
    mm_dt = bf16 if is_bf16x3 else getattr(mybir.dt, matmul_dtype_name)
    i32 = mybir.dt.int32
    AF = mybir.ActivationFunctionType
    ALU = mybir.AluOpType

    any_stt = any(p == "s" for p in pack)

    nc = bacc.Bacc(
        "TRN2",
        target_bir_lowering=False,
        debug=False,
        enable_asserts=False,
        num_devices=NCORES,
    )

    NS = 2 if is_bf16x3 else 1
    xT_d = nc.dram_tensor("xT", (NS, 2, 128, NTOK), mm_dt, kind="ExternalInput").ap()
    cb_d = nc.dram_tensor("cbT2", (NS, 2, 128, K), mm_dt, kind="ExternalInput").ap()
    xsq_d = nc.dram_tensor("xsqp", (128, NTILES), f32, kind="ExternalInput").ap()
    base_d = nc.dram_tensor("base", (128, NTILES), f32, kind="ExternalInput").ap()
    scal_d = nc.dram_tensor("scal", (128, NTILES), f32, kind="ExternalInput").ap()
    nbs_d = nc.dram_tensor("nbaseS", (128, NTILES), f32, kind="ExternalInput").ap()
    iot_d = nc.dram_tensor(
        "iotas", (128, SECW + 2 * TCOLS), f32, kind="ExternalInput"
    ).ap()
    pmul_d = nc.dram_tensor("pmul", (128, NTILES), f32, kind="ExternalInput").ap()
    scalb_d = nc.dram_tensor("scalb", (128, TCOLS), f32, kind="ExternalInput").ap()
    out_d = nc.dram_tensor("idx", (NTILES, 128, 1), i32, kind="ExternalOutput").ap()

    with tile.TileContext(nc) as tc:
        with ExitStack() as ctx:
            cbp = ctx.enter_context(tc.tile_pool(name="cb", bufs=1))
            xp = ctx.enter_context(tc.tile_pool(name="x", bufs=1))
            sp = ctx.enter_context(tc.tile_pool(name="slab", bufs=4))
            x2p = ctx.enter_context(tc.tile_pool(name="x2", bufs=2))
            pp = ctx.enter_context(tc.tile_pool(name="psum", bufs=8, space="PSUM"))
            smp = ctx.enter_context(tc.tile_pool(name="small", bufs=2))
            outp = ctx.enter_context(tc.tile_pool(name="outs", bufs=2))

            xts = {}
            for hl in range(NS):
                for cs in range(2):
                    xt = xp.tile([128, NTOK], mm_dt, tag=f"x{hl}_{cs}")
                    nc.sync.dma_start(xt[:], xT_d[hl][cs])
                    xts[hl, cs] = xt
            xsq = xp.tile([128, NTILES], f32, tag="xsq")
            nc.sync.dma_start(xsq[:], xsq_d[:])
            base = xp.tile([128, NTILES], f32, tag="base")
            nc.sync.dma_start(base[:], base_d[:])
            scal = xp.tile([128, NTILES], f32, tag="scal")
            nc.sync.dma_start(scal[:], scal_d[:])
            nbs = xp.tile([128, NTILES], f32, tag="nbs")
            nc.sync.dma_start(nbs[:], nbs_d[:])
            pmul = xp.tile([128, NTILES], f32, tag="pmul")
            nc.sync.dma_start(pmul[:], pmul_d[:])
            scalb = xp.tile([128, TCOLS], f32, tag="scalb")
            nc.sync.dma_start(scalb[:], scalb_d[:])

            cbs = {}
            for hl in range(NS):
                for cs in range(2):
                    cbt = cbp.tile([128, K], mm_dt, tag=f"cb{hl}_{cs}")
                    nc.sync.dma_start(cbt[:], cb_d[hl][cs])
                    cbs[hl, cs] = cbt

            iotas = xp.tile([128, SECW + 2 * TCOLS], f32, tag="iotas")
            nc.sync.dma_start(iotas[:], iot_d[:])
            iota_sec = iotas[:, 0:SECW]
            iota_s64 = iotas[:, SECW : SECW + TCOLS]
            iota_w64 = iotas[:, SECW + TCOLS : SECW + 2 * TCOLS]
            c_klo = xp.tile([128, TCOLS], i32, tag="c_klo")
            nc.vector.memset(c_klo[:], SECW - 1)
            c_khi = xp.tile([128, TCOLS], i32, tag="c_khi")
            nc.vector.memset(c_khi[:], -SECW)
            c_s = xp.tile([128, NTILES], i32, tag="c_s")
            nc.vector.memset(c_s[:], NSECT - 1)

            if is_bf16x3:
                TERMS = [(0, 0, 0), (0, 0, 1), (0, 1, 0), (0, 1, 1), (1, 0, 0), (1, 0, 1)]
            else:
                TERMS = [(0, 0, 0), (0, 0, 1)]

            for rep in range(repeats):
                minik = smp.tile([128, TCOLS], f32, tag="minik")
                for t in range(NTILES):
                    if any_stt:
                        # X2_t[p,k] = k * ulp_t[p]/2048, exact pow2*int
                        x2t = x2p.tile([128, SECW], f32, tag="x2t")
                        nc.vector.tensor_scalar(
                            x2t[:],
                            iota_sec,
                            pmul[:, t : t + 1],
                            None,
                            op0=ALU.mult,
                        )
                    for sec in range(NSECT):
                        col = t * NSECT + sec
                        pss = [
                            pp.tile(
                                [128, CHUNK], f32, tag="ps",
                                name=f"ps_{rep}_{t}_{sec}_{ci}",
                            )
                            for ci in range(SECW // CHUNK)
                        ]
                        # chunk-major term order: each PSUM chunk retires
                        # after its own TERMS pass, so ACT starts early
                        for ci in range(SECW // CHUNK):
                            k0 = sec * SECW + ci * CHUNK
                            for ti, (xhl, ehl, cs) in enumerate(TERMS):
                                nc.tensor.matmul(
                                    pss[ci][:],
                                    xts[xhl, cs][:, t * 128 : (t + 1) * 128],
                                    cbs[ehl, cs][:, k0 : k0 + CHUNK],
                                    start=(ti == 0),
                                    stop=(ti == len(TERMS) - 1),
                                )
                        slab = sp.tile([128, SECW], f32, tag="slab")
                        # d = fl(xsq - 2mm); per-bank ACT so each chunk's d
                        # starts as soon as its matmul pair retires
                        for ci in range(SECW // CHUNK):
                            nc.scalar.activation(
                                slab[:, ci * CHUNK : (ci + 1) * CHUNK],
                                pss[ci][:],
                                AF.Identity,
                                bias=xsq[:, t : t + 1],
                                scale=-1.0,
                            )
                        if pack[sec] == "s":
                            # fused keys = (d - base) + k*ulp/2048 (exact)
                            nc.vector.scalar_tensor_tensor(
                                slab[:],
                                slab[:],
                                base[:, t : t + 1],
                                x2t[:],
                                op0=ALU.subtract,
                                op1=ALU.add,
                            )
                        else:
                            # ACT: keys0 = S*d + (-S*base), exact; then +iota
                            nc.scalar.activation(
                                slab[:],
                                slab[:],
                                AF.Identity,
                                bias=nbs[:, t : t + 1],
                                scale=scal[:, t : t + 1],
                            )
                            eng = nc.vector if pack[sec] == "A" else nc.gpsimd
                            eng.tensor_tensor(
                                slab[:], slab[:], iota_sec, op=ALU.add
                            )
                        nc.vector.tensor_reduce(
                            minik[:, col : col + 1],
                            slab[:],
                            axis=mybir.AxisListType.X,
                            op=ALU.min,
                        )

                # ---- batched decode over all 64 (tile,sec) minima ----
                if any_stt:
                    # rescale "s" columns to integer keys; exact pow2 for
                    # "s", and for A/P columns scalb holds 1.0
                    nc.vector.tensor_tensor(
                        minik[:], minik[:], scalb[:], op=ALU.mult
                    )
                minik_i = smp.tile([128, TCOLS], i32, tag="minik_i")
                nc.vector.tensor_copy(minik_i[:], minik[:])
                kmod_i = smp.tile([128, TCOLS], i32, tag="kmod_i")
                nc.vector.tensor_tensor(
                    kmod_i[:], minik_i[:], c_klo[:], op=ALU.bitwise_and
                )
                kmod = smp.tile([128, TCOLS], f32, tag="kmod")
                nc.vector.tensor_copy(kmod[:], kmod_i[:])
                dqw_i = smp.tile([128, TCOLS], i32, tag="dqw_i")
                nc.vector.tensor_tensor(
                    dqw_i[:], minik_i[:], c_khi[:], op=ALU.bitwise_and
                )
                dqw = smp.tile([128, TCOLS], f32, tag="dqw")
                nc.vector.tensor_copy(dqw[:], dqw_i[:])
                keys2 = smp.tile([128, TCOLS], f32, tag="keys2")
                nc.vector.tensor_scalar(
                    keys2[:], dqw[:], float(NSECT) / float(SECW), None, op0=ALU.mult
                )
                nc.vector.tensor_tensor(keys2[:], keys2[:], iota_s64, op=ALU.add)
                kfull = smp.tile([128, TCOLS], f32, tag="kfull")
                nc.vector.tensor_tensor(kfull[:], iota_w64, kmod[:], op=ALU.add)
                m2 = smp.tile([128, NTILES], f32, tag="m2")
                nc.vector.tensor_reduce(
                    m2[:],
                    keys2[:].rearrange("p (t s) -> p t s", s=NSECT),
                    axis=mybir.AxisListType.X,
                    op=ALU.min,
                )
                m2i = smp.tile([128, NTILES], i32, tag="m2i")
                nc.vector.tensor_copy(m2i[:], m2[:])
                sstar_i = smp.tile([128, NTILES], i32, tag="sstar_i")
                nc.vector.tensor_tensor(
                    sstar_i[:], m2i[:], c_s[:], op=ALU.bitwise_and
                )
                sstar = smp.tile([128, NTILES], f32, tag="sstar")
                nc.vector.tensor_copy(sstar[:], sstar_i[:])
                mask = smp.tile([128, TCOLS], f32, tag="mask")
                nc.vector.tensor_tensor(
                    mask[:].rearrange("p (t s) -> p t s", s=NSECT),
                    iota_s64.rearrange("p (t s) -> p t s", s=NSECT),
                    sstar[:].unsqueeze(2).broadcast_to([128, NTILES, NSECT]),
                    op=ALU.is_equal,
                )
                nc.vector.tensor_tensor(kfull[:], kfull[:], mask[:], op=ALU.mult)
                kwin = outp.tile([128, NTILES], f32, tag="kwin")
                nc.vector.tensor_reduce(
                    kwin[:],
                    kfull[:].rearrange("p (t s) -> p t s", s=NSECT),
                    axis=mybir.AxisListType.X,
                    op=ALU.add,
                )
                winI = outp.tile([128, NTILES], i32, tag="winI")
                nc.vector.tensor_copy(winI[:], kwin[:])
                for t in range(NTILES):
                    nc.sync.dma_start(out_d[t], winI[:, t : t + 1])

    nc.compile()
    return nc


def get_nc(matmul_dtype_name="float32r", repeats=1, pack=None):
    if pack is None:
        pack = PACK
    key = ("nc", matmul_dtype_name, repeats, tuple(pack))
    if key not in _CACHE:
        _CACHE[key] = _build_bass(matmul_dtype_name, repeats, tuple(pack))
    return _CACHE[key]


def prepare_inputs(hidden_state, codebook, mode="float32r", pack=None):
    """Host-side shard prep: returns in_maps (list of 8 dicts)."""
    import ml_dtypes

    if pack is None:
        pack = PACK
    hs = np.ascontiguousarray(np.asarray(hidden_state, dtype=np.float32))
    cb = np.ascontiguousarray(np.asarray(codebook, dtype=np.float32))
    xT = hs.reshape(B, C, NTOK)
    cb2 = (2.0 * cb.T).astype(np.float32)
    if mode == "bf16x3":
        cb2h = cb2.astype(ml_dtypes.bfloat16)
        cb2l = (cb2 - cb2h.astype(np.float32)).astype(ml_dtypes.bfloat16)
        cb_in = np.ascontiguousarray(np.stack([cb2h, cb2l]).reshape(2, 2, 128, K))
    else:
        cb_in = np.ascontiguousarray(cb2.reshape(1, 2, 128, K))

    sec_ids = np.arange(TCOLS, dtype=np.float32) % NSECT
    iota_row = np.concatenate(
        [np.arange(SECW, dtype=np.float32), sec_ids, sec_ids * SECW]
    )
    iotas = np.ascontiguousarray(np.broadcast_to(iota_row, (128, iota_row.size)))

    emax = float(np.max(np.linalg.norm(2.0 * cb.astype(np.float64), axis=1)))

    # decode rescale: "s" sections produced keys in ulp/2048 units
    s_mask = np.array([1.0 if pack[s] == "s" else 0.0 for s in range(NSECT)],
                      dtype=np.float32)

    in_maps = []
    for b in range(B):
        xb32 = xT[b]
        if mode == "bf16x3":
            xh = xb32.astype(ml_dtypes.bfloat16)
            xl = (xb32 - xh.astype(np.float32)).astype(ml_dtypes.bfloat16)
            xin = np.ascontiguousarray(np.stack([xh, xl]).reshape(2, 2, 128, NTOK))
        else:
            xin = np.ascontiguousarray(xb32.reshape(1, 2, 128, NTOK))
        xsq = np.sum(xb32 * xb32, axis=0, dtype=np.float32)

        xsq64 = xsq.astype(np.float64)
        bound = np.sqrt(xsq64) * emax * 1.2 + 1e-6
        base = (xsq64 - bound).astype(np.float32)
        _, exp = np.frexp(base)
        ulp = np.ldexp(np.float64(1.0), exp - 24)
        dq_max = (xsq64 + bound - base.astype(np.float64)) / ulp
        assert (base > 0).all() and (dq_max < 8100).all(), (
            f"distance-spread exceeds 13-bit key budget; max dq={dq_max.max():.0f}"
        )
        scal = np.ldexp(np.float32(SECW), -(exp - 24)).astype(np.float32)
        nbaseS = (-(base.astype(np.float64) * scal.astype(np.float64))).astype(
            np.float32
        )

        def pt(a):
            return np.ascontiguousarray(a.reshape(NTILES, 128).T)

        scal_t = pt(scal)
        # scalb: per minik column; scal for "s" sections, 1.0 for A/P
        scalb = np.repeat(scal_t, NSECT, axis=1) * s_mask[None, :].repeat(
            NTILES, axis=0
        ).reshape(1, TCOLS)
        scalb = (scalb + (1.0 - np.tile(s_mask, NTILES))[None, :]).astype(np.float32)

        in_maps.append(
            {
                "xT": xin,
                "cbT2": cb_in,
                "xsqp": pt(xsq),
                "base": pt(base),
                "scal": scal_t,
                "nbaseS": pt(nbaseS),
                "iotas": iotas,
                "pmul": np.ascontiguousarray(1.0 / scal_t),
                "scalb": np.ascontiguousarray(scalb),
            }
        )
    return in_maps


MODE = "float32r"
# per-section key engine: "s"=DVE fused stt, "A"=ACT pack + DVE iota-add,
# "P"=ACT pack + GpSimd iota-add
PACK = ("s", "s", "s", "s", "P", "P", "P", "P")


def kernel(hidden_state, codebook):
    from concourse.bass_utils import run_bass_kernel_spmd

    nc = get_nc(MODE, 1, PACK)
    in_maps = prepare_inputs(hidden_state, codebook, MODE, PACK)
    res = run_bass_kernel_spmd(nc, in_maps, core_ids=list(range(NCORES)))
    out = np.stack(
        [res.results[b]["idx"].reshape(NTOK) for b in range(B)], axis=0
    ).astype(np.int32)
    return out.reshape(B, T, H, W)
